# revision 4
# baseline (speedup 1.0000x reference)
"""Trainium2 Bass kernel for two-stream cross-attention (v6).

Reference computation (per batch b):
    qkv_s = x_s @ W_qkv_s ; split into q_s, k_s, v_s (16 heads x 64)
    dir1: out1 = softmax(q2 k1^T * scale) v1, merged @ W_out1 + b_out1
    dir2: out2 = softmax(q1 k2^T * scale) v2, merged @ W_out2 + b_out2

Sharding: 8 cores = 2 batches x 4 head-groups (4 heads each). Each core
computes q/k/v for its 4 heads (both streams), both attention directions,
and a partial output projection (row-block of W_out). Host transposes x
(so the device only does linear DMA) and sums the 4 f16 partials per
batch, adding the bias.

v6 changes vs v5:
  - AV matmuls are v-stationary: out O^T[65, 512] accumulates over key
    blocks with the P tile as the N=512 moving operand (512 matmuls of
    N=512 instead of 2048 of N=65; no LDWEIGHTS bottleneck). The 65th
    row of O^T is the softmax denominator (ones column in v).
  - O^T normalization: reciprocal of the denom row -> rank-1 K=1 matmul
    broadcasts it across 64 partitions -> one DVE multiply writes the
    normalized O^T directly into ot. The 128 PE transposes, per-m-block
    reciprocal/scale and ot copies of v5 are gone.
  - Warmup matmuls 450 -> 120 (v5's 450 delayed the first real matmul
    ~25us past the DMA window).
  - wo DMA moved after the x/w loads (it blocked the critical x path).
  - Head fills reordered: qT1 nt0 only, then kT0/v0 nt-interleaved, so
    dir0 attention starts as soon as the DMA window closes; qT1 nt1-3
    moved to the low-priority fill stream.
"""

import os

import numpy as np
import ml_dtypes

import concourse.bass as bass
import concourse.mybir as mybir
import concourse.tile as tile
from concourse import bacc
from concourse.bass_utils import run_bass_kernel_spmd
from concourse.masks import make_identity

BF16 = mybir.dt.bfloat16
F16 = mybir.dt.float16
F32 = mybir.dt.float32

B, N, DIM = 2, 2048, 1024
HEADS, DH = 16, 64
HPC = 4                      # heads per core
HC = HPC * DH                # 256 inner columns per core
SCALE = DH ** -0.5
P = 128
FB = DIM // P                # 8 feature blocks
KB = N // P                  # 16 key blocks
QT = 512                     # q-tile
NQT = N // QT                # 4 q-tiles
NM = QT // P                 # 4 m-blocks per q-tile

NCORES = 8
LOWPRI = 10_000_000          # negative high_priority offset for fill work

_NC = None
LAST_RESULTS = None


def _build():
    nc = bacc.Bacc(None, target_bir_lowering=False, debug=False, num_devices=NCORES)

    # x is pre-transposed on the host: x^T [DIM, N] -> linear DMA loads
    xs = [nc.dram_tensor(f"x{s + 1}", [DIM, N], BF16, kind="ExternalInput")
          for s in range(2)]
    ws = [nc.dram_tensor(f"w{s + 1}", [DIM, 3 * HC], BF16, kind="ExternalInput")
          for s in range(2)]
    wos = [nc.dram_tensor(f"wo{s + 1}", [HC, DIM], BF16, kind="ExternalInput")
           for s in range(2)]
    os_ = [nc.dram_tensor(f"o{d + 1}", [N, DIM], F16, kind="ExternalOutput")
           for d in range(2)]

    with tile.TileContext(nc) as tc:
        with (
            tc.tile_pool(name="const", bufs=1) as const_pool,
            tc.tile_pool(name="qkv", bufs=1) as qkv_pool,
        ):
            identity = const_pool.tile([P, P], BF16)
            make_identity(nc, identity[:])
            ones = const_pool.tile([1, DH], F32, name="ones")
            nc.vector.memset(ones[:], 1.0)
            wo_sb = [const_pool.tile([P, 2, DIM], BF16, name=f"wo{d}")
                     for d in range(2)]

            # persistent per-stream q/k/v (bf16) and per-dir O^T
            qT = [qkv_pool.tile([P, 2, N], BF16, name=f"qT{s}") for s in range(2)]
            kT = [qkv_pool.tile([P, 2, N], BF16, name=f"kT{s}") for s in range(2)]
            vx = [qkv_pool.tile([P, KB, HPC, DH + 1], BF16, name=f"vx{s}")
                  for s in range(2)]
            ot = [qkv_pool.tile([P, 2, N], BF16, name=f"ot{d}") for d in range(2)]
            for s in range(2):
                nc.vector.memset(vx[s][:, :, :, DH], 1.0)

            with (
                tc.tile_pool(name="xT", bufs=1) as xt_pool,
                tc.tile_pool(name="wsb", bufs=1) as w_pool,
                tc.tile_pool(name="pmm", bufs=1, space="PSUM") as pmm_pool,
                tc.tile_pool(name="st", bufs=2, space="PSUM") as st_pool,
                tc.tile_pool(name="oav", bufs=2, space="PSUM") as oav_pool,
                tc.tile_pool(name="ptrpop", bufs=1, space="PSUM") as ptr_pool,
                tc.tile_pool(name="pt", bufs=6) as pt_pool,
                tc.tile_pool(name="rec", bufs=4) as rec_pool,
                tc.tile_pool(name="bcs", bufs=2) as bcs_pool,
                tc.tile_pool(name="ost", bufs=3) as ost_pool,
            ):
                # HAM warmup: no-dep dummy matmuls, first in the PE queue.
                # Enough to hold the PE busy through the HAM SHORT window
                # (~3.4us) plus the head-DMA window, no more.
                warm = pmm_pool.tile([P, P], F32, name="warm", tag="pmm")
                for _ in range(120):
                    nc.tensor.matmul(warm[:], identity[:], identity[:],
                                     start=True, stop=True)

                xT = [xt_pool.tile([P, FB, N], BF16, name=f"xT{s}")
                      for s in range(2)]
                w_sb = [w_pool.tile([P, FB, 3 * HC], BF16, name=f"w{s}")
                        for s in range(2)]
                # interleave both streams' loads across the DMA queues;
                # stream 1 slightly first (dir0 needs qT[1] + kT[0])
                for fb in range(FB):
                    for s in (1, 0):
                        nc.sync.dma_start(
                            w_sb[s][:, fb, :], ws[s][fb * P:(fb + 1) * P, :])
                        nc.sync.dma_start(
                            xT[s][:, fb, :], xs[s][fb * P:(fb + 1) * P, :])
                # wo is first needed by the qt0 output projection (~60us in)
                for d in range(2):
                    for cb in range(2):
                        nc.sync.dma_start(
                            wo_sb[d][:, cb, :], wos[d][cb * P:(cb + 1) * P, :])

                def qk_group(s, off, cb, nt, dest, pool, tag):
                    ps = pool.tile([P, 512], F32, name="pqk", tag=tag)
                    for fb in range(FB):
                        nc.tensor.matmul(
                            ps[:],
                            w_sb[s][:, fb, off + cb * P:off + (cb + 1) * P],
                            xT[s][:, fb, nt * 512:(nt + 1) * 512],
                            start=(fb == 0), stop=(fb == FB - 1))
                    nc.vector.tensor_copy(dest[:, cb, nt * 512:(nt + 1) * 512],
                                          ps[:])

                def v_group(s, kb, pool, tag):
                    ps = pool.tile([P, HC], F32, name="pv", tag=tag)
                    for fb in range(FB):
                        nc.tensor.matmul(
                            ps[:],
                            xT[s][:, fb, kb * P:(kb + 1) * P],
                            w_sb[s][:, fb, 2 * HC:3 * HC],
                            start=(fb == 0), stop=(fb == FB - 1))
                    nc.vector.tensor_copy(
                        vx[s][:, kb, :, 0:DH],
                        ps[:].rearrange("p (h d) -> p h d", h=HPC))

                # ---- head: the minimum qkv for dir0's first unit ----
                # dir0 unit(qt=0) needs qT1 nt0, all of kT0 and all of vx0
                # (kb-granular). Alternate the two idle PSUM banks (pmm,
                # ptrpop) for a 2-deep fill pipeline. st/oav stay
                # attention-only so their FIFO slot grants aren't delayed.
                hp = [(pmm_pool, "pmm"), (ptr_pool, "ptrpop")]
                hi = 0

                def nxt():
                    nonlocal hi
                    pl = hp[hi % 2]
                    hi += 1
                    return pl

                for cb in range(2):
                    pl = nxt()
                    qk_group(1, 0, cb, 0, qT[1], pl[0], pl[1])
                for nt in range(4):
                    for cb in range(2):
                        pl = nxt()
                        qk_group(0, HC, cb, nt, kT[0], pl[0], pl[1])
                    for kb in range(4 * nt, 4 * nt + 4):
                        pl = nxt()
                        v_group(0, kb, pl[0], pl[1])

                # ---- remaining qkv at very low scheduler preference:
                # executes only in PE gaps of the Scalar-bound attention
                # stream. Order staged so dir1's earliest needs (kT1/qT0
                # nt0, first v1 blocks) come first, then dir0's qT1 nt1-3
                # (needed at qt=1,2,3 of dir0).
                with tc.high_priority(offset=-LOWPRI):
                    for cb in range(2):
                        qk_group(1, 0, cb, 1, qT[1], pmm_pool, "pmm")
                    for cb in range(2):
                        qk_group(0, 0, cb, 0, qT[0], pmm_pool, "pmm")
                        qk_group(1, HC, cb, 0, kT[1], pmm_pool, "pmm")
                    for nt in range(2, 4):
                        for cb in range(2):
                            qk_group(1, 0, cb, nt, qT[1], pmm_pool, "pmm")
                    for nt in range(4):
                        for kb in range(4 * nt, 4 * nt + 4):
                            v_group(1, kb, pmm_pool, "pmm")
                        if nt > 0:
                            for cb in range(2):
                                qk_group(1, HC, cb, nt, kT[1], pmm_pool, "pmm")
                    for nt in range(1, 4):
                        for cb in range(2):
                            qk_group(0, 0, cb, nt, qT[0], pmm_pool, "pmm")

                def attn_unit(d, qs, ks, qt, cb):
                    """Head pair (2*cb, 2*cb+1), queries qt*QT..+QT.

                    Per key block: S^T pair (row-tiled K=64 concurrent
                    matmuls) -> one exp -> per head one v-stationary AV
                    matmul accumulating O^T[65, QT] (row 64 = denom).
                    Then per head: reciprocal -> K=1 rank-1 broadcast
                    matmul -> DVE multiply writes normalized O^T into ot.
                    """
                    q_t, k_t, v_t = qT[qs], kT[ks], vx[ks]
                    q0 = qt * QT
                    oT = [oav_pool.tile([DH + 1, QT], F32, name="oT",
                                        tag="oav")
                          for _ in range(2)]
                    for kb in range(KB):
                        st = st_pool.tile([P, 2, QT], F32, name="st", tag="st")
                        for hh in range(2):
                            po = hh * DH
                            nc.tensor.matmul(
                                st[:, hh, :],
                                k_t[po:po + DH, cb, kb * P:(kb + 1) * P],
                                q_t[po:po + DH, cb, q0:q0 + QT],
                                start=True, stop=True)
                        pt = pt_pool.tile([P, 2, QT], BF16, name="pt")
                        nc.scalar.activation(
                            pt[:], st[:],
                            mybir.ActivationFunctionType.Exp, scale=SCALE)
                        for hh in range(2):
                            head = 2 * cb + hh
                            nc.tensor.matmul(
                                oT[hh][:, :],
                                v_t[:, kb, head, :],
                                pt[:, hh, :],
                                start=(kb == 0), stop=(kb == KB - 1),
                                skip_group_check=True)
                    for hh in range(2):
                        po = hh * DH
                        rec = rec_pool.tile([1, QT], F32, name="rec")
                        nc.vector.reciprocal(rec[:], oT[hh][DH:DH + 1, :])
                        bcr = ptr_pool.tile([DH, QT], F32, name="bcr",
                                            tag="ptrpop")
                        nc.tensor.matmul(bcr[:], ones[:], rec[:],
                                         start=True, stop=True)
                        bcs = bcs_pool.tile([DH, QT], BF16, name="bcs")
                        nc.vector.tensor_copy(bcs[:], bcr[:])
                        nc.vector.tensor_mul(
                            ot[d][po:po + DH, cb, q0:q0 + QT],
                            oT[hh][0:DH, :], bcs[:])

                def outproj(d, qt):
                    for mb in range(NM):
                        row = qt * QT + mb * P
                        ost = ost_pool.tile([P, DIM], F16, name="ost")
                        for nb in range(2):
                            pop = ptr_pool.tile([P, 512], F32, name="pop",
                                                tag="ptrpop")
                            for cb2 in range(2):
                                nc.tensor.matmul(
                                    pop[:],
                                    ot[d][:, cb2, row:row + P],
                                    wo_sb[d][:, cb2, nb * 512:(nb + 1) * 512],
                                    start=(cb2 == 0), stop=(cb2 == 1))
                            nc.vector.tensor_copy(ost[:, nb * 512:(nb + 1) * 512],
                                                  pop[:])
                        nc.sync.dma_start(os_[d][row:row + P, :], ost[:])

                # ---- attention + inline output projection ----
                for qt in range(NQT):
                    for cb in range(2):
                        attn_unit(0, 1, 0, qt, cb)
                    outproj(0, qt)
                for qt in range(NQT):
                    for cb in range(2):
                        attn_unit(1, 0, 1, qt, cb)
                    outproj(1, qt)

    nc.compile()
    return nc


def _shard_inputs(x1, x2, W_qkv1, W_qkv2, W_out1, W_out2):
    bf = ml_dtypes.bfloat16
    in_maps = []
    xs = [np.ascontiguousarray(x1).astype(bf), np.ascontiguousarray(x2).astype(bf)]
    w_full = [np.asarray(W_qkv1), np.asarray(W_qkv2)]
    wo_full = [np.asarray(W_out1), np.asarray(W_out2)]
    for cid in range(NCORES):
        b, g = divmod(cid, 4)
        cs = slice(g * HC, (g + 1) * HC)
        m = {}
        for s in range(2):
            m[f"x{s + 1}"] = np.ascontiguousarray(xs[s][b].T)
            w = w_full[s]
            m[f"w{s + 1}"] = np.ascontiguousarray(np.concatenate(
                [w[:, 0:DIM][:, cs], w[:, DIM:2 * DIM][:, cs],
                 w[:, 2 * DIM:3 * DIM][:, cs]], axis=1)).astype(bf)
            m[f"wo{s + 1}"] = np.ascontiguousarray(wo_full[s][cs, :]).astype(bf)
        in_maps.append(m)
    return in_maps


def kernel(x1, x2, W_qkv1, W_qkv2, W_out1, b_out1, W_out2, b_out2):
    global _NC, LAST_RESULTS
    if _NC is None:
        _NC = _build()

    in_maps = _shard_inputs(x1, x2, W_qkv1, W_qkv2, W_out1, W_out2)
    trace = bool(os.environ.get("BASS_KERNEL_TRACE"))
    res = run_bass_kernel_spmd(_NC, in_maps, list(range(NCORES)), trace=trace)
    LAST_RESULTS = res

    out1 = np.zeros((B, N, DIM), np.float32)
    out2 = np.zeros((B, N, DIM), np.float32)
    for cid in range(NCORES):
        b = cid // 4
        out1[b] += res.results[cid]["o1"].astype(np.float32)
        out2[b] += res.results[cid]["o2"].astype(np.float32)
    out1 += np.asarray(b_out1, np.float32)
    out2 += np.asarray(b_out2, np.float32)
    return out1, out2


# revision 6
# speedup vs baseline: 1.4226x; 1.4226x over previous
"""Trainium2 Bass kernel for two-stream cross-attention (v7).

Reference computation (per batch b):
    qkv_s = x_s @ W_qkv_s ; split into q_s, k_s, v_s (16 heads x 64)
    dir1: out1 = softmax(q2 k1^T * scale) v1, merged @ W_out1 + b_out1
    dir2: out2 = softmax(q1 k2^T * scale) v2, merged @ W_out2 + b_out2

Sharding: 8 cores = 2 batches x 4 head-groups (4 heads each). Each core
computes q/k/v for its 4 heads (both streams), both attention directions,
and a partial output projection (row-block of W_out). Host transposes x
(so the device only does linear DMA) and sums the 4 f16 partials per
batch, adding the bias.

Kernel structure (all matmuls bf16, fp32 PSUM accumulation):
  - Heads processed in row-tiled PAIRS: head 2*cb on partitions 0-63,
    head 2*cb+1 on 64-127. The pair's two S^T matmuls (K=64) carry
    tile_position (0,0)/(64,0), run CONCURRENTLY in the PE array into
    different PSUM banks -> 2x S throughput.
  - Flash-style inner loop per key block: S-pair -> one exp[128,1024]
    covering both heads -> 8 AV matmuls accumulating into per-head
    oav[128,4,72] PSUM banks (appended ones-column = softmax rowsum).
  - ScalarE (exp) is the critical engine (~285us busy). The PE's
    attention work (small AV matmuls, transposes, outproj) doubles as
    HAM activity keeping the PE clock at 2.4 GHz; a leaner AV structure
    (v6) measured WORSE because the PE throttled to 1.2 GHz and became
    the bottleneck.
  - v7 head: only dir0-critical DMA early (x1+x2, w1 k/v cols, w2 q
    cols; dir1 w cols and wo deferred), and the FIRST attention unit is
    striped into the kT0/v0 fill chunks so exp starts as soon as the
    x DMA lands instead of after all fills.
  - PSUM pools: st 4 banks, oav 2 (attention-only), ptr/outproj+head
    fills 1, pmm fills 1.
"""

import os

import numpy as np
import ml_dtypes

import concourse.bass as bass
import concourse.mybir as mybir
import concourse.tile as tile
from concourse import bacc
from concourse.bass_utils import run_bass_kernel_spmd
from concourse.masks import make_identity

BF16 = mybir.dt.bfloat16
F16 = mybir.dt.float16
F32 = mybir.dt.float32

B, N, DIM = 2, 2048, 1024
HEADS, DH = 16, 64
HPC = 4                      # heads per core
HC = HPC * DH                # 256 inner columns per core
SCALE = DH ** -0.5
P = 128
FB = DIM // P                # 8 feature blocks
KB = N // P                  # 16 key blocks
QT = 512                     # q-tile
NQT = N // QT                # 4 q-tiles
NM = QT // P                 # 4 m-blocks per q-tile

NCORES = 8
LOWPRI = 10_000_000          # negative high_priority offset for fill work

_NC = None
LAST_RESULTS = None


def _build():
    nc = bacc.Bacc(None, target_bir_lowering=False, debug=False, num_devices=NCORES)

    # x is pre-transposed on the host: x^T [DIM, N] -> linear DMA loads
    xs = [nc.dram_tensor(f"x{s + 1}", [DIM, N], BF16, kind="ExternalInput")
          for s in range(2)]
    ws = [nc.dram_tensor(f"w{s + 1}", [DIM, 3 * HC], BF16, kind="ExternalInput")
          for s in range(2)]
    wos = [nc.dram_tensor(f"wo{s + 1}", [HC, DIM], BF16, kind="ExternalInput")
           for s in range(2)]
    os_ = [nc.dram_tensor(f"o{d + 1}", [N, DIM], F16, kind="ExternalOutput")
           for d in range(2)]

    with tile.TileContext(nc) as tc:
        with (
            tc.tile_pool(name="const", bufs=1) as const_pool,
            tc.tile_pool(name="qkv", bufs=1) as qkv_pool,
        ):
            identity = const_pool.tile([P, P], BF16)
            make_identity(nc, identity[:])
            wo_sb = [const_pool.tile([P, 2, DIM], BF16, name=f"wo{d}")
                     for d in range(2)]

            # persistent per-stream q/k/v (bf16) and per-dir O^T
            qT = [qkv_pool.tile([P, 2, N], BF16, name=f"qT{s}") for s in range(2)]
            kT = [qkv_pool.tile([P, 2, N], BF16, name=f"kT{s}") for s in range(2)]
            vx = [qkv_pool.tile([P, KB, HPC, DH + 1], BF16, name=f"vx{s}")
                  for s in range(2)]
            ot = [qkv_pool.tile([P, 2, N], BF16, name=f"ot{d}") for d in range(2)]
            for s in range(2):
                nc.vector.memset(vx[s][:, :, :, DH], 1.0)

            with (
                tc.tile_pool(name="xT", bufs=1) as xt_pool,
                tc.tile_pool(name="wsb", bufs=1) as w_pool,
                tc.tile_pool(name="pmm", bufs=1, space="PSUM") as pmm_pool,
                tc.tile_pool(name="st", bufs=2, space="PSUM") as st_pool,
                tc.tile_pool(name="oav", bufs=2, space="PSUM") as oav_pool,
                tc.tile_pool(name="ptrpop", bufs=1, space="PSUM") as ptr_pool,
                tc.tile_pool(name="pt", bufs=6) as pt_pool,
                tc.tile_pool(name="osb", bufs=4) as osb_pool,
                tc.tile_pool(name="rec", bufs=4) as rec_pool,
                tc.tile_pool(name="ost", bufs=3) as ost_pool,
            ):
                # HAM warmup: no-dep dummy matmuls, first in the PE queue.
                # Fill matmuls trickle in from ~6us as x fb-slices land,
                # so only the first ~8us needs dummy coverage.
                warm = pmm_pool.tile([P, P], F32, name="warm", tag="pmm")
                for _ in range(100):
                    nc.tensor.matmul(warm[:], identity[:], identity[:],
                                     start=True, stop=True)

                xT = [xt_pool.tile([P, FB, N], BF16, name=f"xT{s}")
                      for s in range(2)]
                w_sb = [w_pool.tile([P, FB, 3 * HC], BF16, name=f"w{s}")
                        for s in range(2)]
                # Critical DMA first: x of both streams + only the w
                # columns dir0 needs (stream1 k/v, stream2 q). dir1's w
                # columns and wo queue up behind them.
                W_EARLY = [(HC, 3 * HC), (0, HC)]
                W_LATE = [(0, HC), (HC, 3 * HC)]
                for fb in range(FB):
                    for s in (1, 0):
                        lo, hi = W_EARLY[s]
                        nc.sync.dma_start(
                            w_sb[s][:, fb, lo:hi],
                            ws[s][fb * P:(fb + 1) * P, lo:hi])
                        nc.sync.dma_start(
                            xT[s][:, fb, :], xs[s][fb * P:(fb + 1) * P, :])
                for fb in range(FB):
                    for s in (1, 0):
                        lo, hi = W_LATE[s]
                        nc.sync.dma_start(
                            w_sb[s][:, fb, lo:hi],
                            ws[s][fb * P:(fb + 1) * P, lo:hi])
                for d in range(2):
                    for cb in range(2):
                        nc.sync.dma_start(
                            wo_sb[d][:, cb, :], wos[d][cb * P:(cb + 1) * P, :])

                def qk_group(s, off, cb, nt, dest, pool, tag):
                    ps = pool.tile([P, 512], F32, name="pqk", tag=tag)
                    for fb in range(FB):
                        nc.tensor.matmul(
                            ps[:],
                            w_sb[s][:, fb, off + cb * P:off + (cb + 1) * P],
                            xT[s][:, fb, nt * 512:(nt + 1) * 512],
                            start=(fb == 0), stop=(fb == FB - 1))
                    nc.vector.tensor_copy(dest[:, cb, nt * 512:(nt + 1) * 512],
                                          ps[:])

                def v_group(s, kb, pool, tag):
                    ps = pool.tile([P, HC], F32, name="pv", tag=tag)
                    for fb in range(FB):
                        nc.tensor.matmul(
                            ps[:],
                            xT[s][:, fb, kb * P:(kb + 1) * P],
                            w_sb[s][:, fb, 2 * HC:3 * HC],
                            start=(fb == 0), stop=(fb == FB - 1))
                    nc.vector.tensor_copy(
                        vx[s][:, kb, :, 0:DH],
                        ps[:].rearrange("p (h d) -> p h d", h=HPC))

                def attn_begin():
                    return [oav_pool.tile([P, NM, 72], F32, name="oav",
                                          tag="oav")
                            for _ in range(2)]

                def attn_chunk(oav, d, qs, ks, qt, cb, kb_lo, kb_hi):
                    q_t, k_t, v_t = qT[qs], kT[ks], vx[ks]
                    q0 = qt * QT
                    for kb in range(kb_lo, kb_hi):
                        st = st_pool.tile([P, 2, QT], F32, name="st", tag="st")
                        for hh in range(2):
                            po = hh * DH
                            nc.tensor.matmul(
                                st[:, hh, :],
                                k_t[po:po + DH, cb, kb * P:(kb + 1) * P],
                                q_t[po:po + DH, cb, q0:q0 + QT],
                                start=True, stop=True)
                        pt = pt_pool.tile([P, 2, QT], BF16, name="pt")
                        nc.scalar.activation(
                            pt[:], st[:],
                            mybir.ActivationFunctionType.Exp, scale=SCALE)
                        for hh in range(2):
                            head = 2 * cb + hh
                            for m in range(NM):
                                nc.tensor.matmul(
                                    oav[hh][:, m, 0:DH + 1],
                                    pt[:, hh, m * P:(m + 1) * P],
                                    v_t[:, kb, head, :],
                                    start=(kb == 0 and m == 0),
                                    stop=(kb == KB - 1 and m == NM - 1),
                                    skip_group_check=True)

                def attn_end(oav, d, qt, cb):
                    # normalize, transpose O -> O^T, write into ot
                    q0 = qt * QT
                    ptr = ptr_pool.tile([DH, 2 * NM, P], BF16, name="ptr",
                                        tag="ptrpop")
                    for hh in range(2):
                        for m in range(NM):
                            rec = rec_pool.tile([P, 1], F32, name="rec")
                            nc.vector.reciprocal(rec[:], oav[hh][:, m, DH:DH + 1])
                            osb = osb_pool.tile([P, DH], BF16, name="osb")
                            nc.vector.tensor_scalar_mul(
                                osb[:], oav[hh][:, m, 0:DH], rec[:])
                            nc.tensor.transpose(
                                ptr[:, hh * NM + m, :], osb[:], identity[:])
                    for hh in range(2):
                        po = hh * DH
                        nc.vector.tensor_copy(
                            ot[d][po:po + DH, cb, q0:q0 + QT],
                            ptr[:, hh * NM:(hh + 1) * NM, :])

                def attn_unit(d, qs, ks, qt, cb):
                    oav = attn_begin()
                    attn_chunk(oav, d, qs, ks, qt, cb, 0, KB)
                    attn_end(oav, d, qt, cb)

                def outproj(d, qt):
                    for mb in range(NM):
                        row = qt * QT + mb * P
                        ost = ost_pool.tile([P, DIM], F16, name="ost")
                        for nb in range(2):
                            pop = ptr_pool.tile([P, 512], F32, name="pop",
                                                tag="ptrpop")
                            for cb2 in range(2):
                                nc.tensor.matmul(
                                    pop[:],
                                    ot[d][:, cb2, row:row + P],
                                    wo_sb[d][:, cb2, nb * 512:(nb + 1) * 512],
                                    start=(cb2 == 0), stop=(cb2 == 1))
                            nc.vector.tensor_copy(ost[:, nb * 512:(nb + 1) * 512],
                                                  pop[:])
                        nc.sync.dma_start(os_[d][row:row + P, :], ost[:])

                # ---- head: dir0's first unit striped into its fills ----
                # Fill groups alternate the pmm/ptrpop banks (2-deep
                # pipeline); st/oav stay attention-only. After each
                # kT0-nt + v0 chunk, the matching kb chunk of unit
                # (dir0, qt0, cb0) is emitted, so exp starts right after
                # the x DMA + ~4 fill groups instead of after all 26.
                hseq = [(pmm_pool, "pmm"), (ptr_pool, "ptrpop")]
                hidx = [0]

                def nxt():
                    pl = hseq[hidx[0] % 2]
                    hidx[0] += 1
                    return pl

                for cb in range(2):
                    pl = nxt()
                    qk_group(1, 0, cb, 0, qT[1], pl[0], pl[1])
                oav0 = attn_begin()
                for nt in range(4):
                    for cb in range(2):
                        pl = nxt()
                        qk_group(0, HC, cb, nt, kT[0], pl[0], pl[1])
                    for kb in range(4 * nt, 4 * nt + 4):
                        pl = nxt()
                        v_group(0, kb, pl[0], pl[1])
                    attn_chunk(oav0, 0, 1, 0, 0, 0, 4 * nt, 4 * nt + 4)
                attn_end(oav0, 0, 0, 0)

                # ---- remaining qkv at very low scheduler preference:
                # executes only in PE gaps of the Scalar-bound attention
                # stream. Ordered by first use: dir0 qt1's queries, then
                # dir1 nt0 k/q, dir0 qt2/3 queries, dir1 v + rest.
                with tc.high_priority(offset=-LOWPRI):
                    for cb in range(2):
                        qk_group(1, 0, cb, 1, qT[1], pmm_pool, "pmm")
                    for cb in range(2):
                        qk_group(0, 0, cb, 0, qT[0], pmm_pool, "pmm")
                        qk_group(1, HC, cb, 0, kT[1], pmm_pool, "pmm")
                    for nt in range(2, 4):
                        for cb in range(2):
                            qk_group(1, 0, cb, nt, qT[1], pmm_pool, "pmm")
                    for nt in range(4):
                        for kb in range(4 * nt, 4 * nt + 4):
                            v_group(1, kb, pmm_pool, "pmm")
                        if nt > 0:
                            for cb in range(2):
                                qk_group(1, HC, cb, nt, kT[1], pmm_pool, "pmm")
                    for nt in range(1, 4):
                        for cb in range(2):
                            qk_group(0, 0, cb, nt, qT[0], pmm_pool, "pmm")

                # ---- attention + inline output projection ----
                attn_unit(0, 1, 0, 0, 1)
                outproj(0, 0)
                for qt in range(1, NQT):
                    for cb in range(2):
                        attn_unit(0, 1, 0, qt, cb)
                    outproj(0, qt)
                for qt in range(NQT):
                    for cb in range(2):
                        attn_unit(1, 0, 1, qt, cb)
                    outproj(1, qt)

    nc.compile()
    return nc


def _shard_inputs(x1, x2, W_qkv1, W_qkv2, W_out1, W_out2):
    bf = ml_dtypes.bfloat16
    in_maps = []
    xs = [np.ascontiguousarray(x1).astype(bf), np.ascontiguousarray(x2).astype(bf)]
    w_full = [np.asarray(W_qkv1), np.asarray(W_qkv2)]
    wo_full = [np.asarray(W_out1), np.asarray(W_out2)]
    for cid in range(NCORES):
        b, g = divmod(cid, 4)
        cs = slice(g * HC, (g + 1) * HC)
        m = {}
        for s in range(2):
            m[f"x{s + 1}"] = np.ascontiguousarray(xs[s][b].T)
            w = w_full[s]
            m[f"w{s + 1}"] = np.ascontiguousarray(np.concatenate(
                [w[:, 0:DIM][:, cs], w[:, DIM:2 * DIM][:, cs],
                 w[:, 2 * DIM:3 * DIM][:, cs]], axis=1)).astype(bf)
            m[f"wo{s + 1}"] = np.ascontiguousarray(wo_full[s][cs, :]).astype(bf)
        in_maps.append(m)
    return in_maps


def kernel(x1, x2, W_qkv1, W_qkv2, W_out1, b_out1, W_out2, b_out2):
    global _NC, LAST_RESULTS
    if _NC is None:
        _NC = _build()

    in_maps = _shard_inputs(x1, x2, W_qkv1, W_qkv2, W_out1, W_out2)
    trace = bool(os.environ.get("BASS_KERNEL_TRACE"))
    res = run_bass_kernel_spmd(_NC, in_maps, list(range(NCORES)), trace=trace)
    LAST_RESULTS = res

    out1 = np.zeros((B, N, DIM), np.float32)
    out2 = np.zeros((B, N, DIM), np.float32)
    for cid in range(NCORES):
        b = cid // 4
        out1[b] += res.results[cid]["o1"].astype(np.float32)
        out2[b] += res.results[cid]["o2"].astype(np.float32)
    out1 += np.asarray(b_out1, np.float32)
    out2 += np.asarray(b_out2, np.float32)
    return out1, out2
